# revision 1
# baseline (speedup 1.0000x reference)
"""Trainium2 Bass kernel for nn_Net_76562087018570.

Computation (reference): per-column MinMax scale of a (4096, 8192) f32 matrix,
10 iterations of arr = arr*(1 - (arr - rowmean(arr))) (+0.001 on iter 0),
then inverse transform.

Strategy: shard rows across 8 cores (512 rows each, 16 MiB -> SBUF resident).
Instead of arr we carry sq_k = (arr_k - h_k)^2 with h_k = (1+mean_k)/2:
    arr_{k+1} = arr_k*(b_k - arr_k) + c_k = h_k^2 + c_k - sq_k   (b=1+mean)
    sq_{k+1}  = (beta_k - sq_k)^2,  beta_k = gamma_k - h_{k+1},
    gamma_k   = h_k^2 + c_k,  mean_{k+1} = gamma_k - rowsum(sq_k)/n
so each iteration is ONE ACT Square pass (pre-affine + fused row-sum accum),
with a column-slice offloaded to DVE (tensor_scalar + tensor_tensor_reduce)
to balance the two engines.  Per-column min/max uses PE transposes into PSUM
+ DVE free-axis reduces; the cross-core reduction is an AllReduce(max) on a
packed [max; -min] (128,128) buffer.
"""

import os
import numpy as np

R = 512          # rows per core
N = 8192         # columns
NT = 4           # (128,N) tiles per core
NCORES = 8
NITERS = 10
ACOL = 5376      # columns handled by ACT in iteration passes (rest on DVE)

_cache = {}
LAST_RESULT = None


def _build():
    import concourse.bacc as bacc
    import concourse.tile as tile
    from concourse import mybir, masks

    f32 = mybir.dt.float32
    A = mybir.AluOpType
    AF = mybir.ActivationFunctionType
    AX = mybir.AxisListType

    nc = bacc.Bacc(trn_type="TRN2", num_devices=NCORES)
    xs = nc.dram_tensor("xs", [R, N], f32, kind="ExternalInput")
    out = nc.dram_tensor("out", [R, N], f32, kind="ExternalOutput")
    xv = xs.ap().rearrange("(t p) n -> t p n", p=128)
    ov = out.ap().rearrange("(t p) n -> t p n", p=128)

    with tile.TileContext(nc) as tc:
        with tc.tile_pool(name="big", bufs=1) as big, \
             tc.tile_pool(name="bcast", bufs=1) as bcp, \
             tc.tile_pool(name="consts", bufs=1) as consts, \
             tc.tile_pool(name="mm", bufs=1) as mm, \
             tc.tile_pool(name="small", bufs=4) as small, \
             tc.tile_pool(name="psum", bufs=2, space="PSUM") as psum, \
             tc.tile_pool(name="psq", bufs=1, space="PSUM") as psq, \
             tc.tile_pool(name="dram", bufs=1, space="DRAM") as dram:

            ident = consts.tile([128, 128], f32)
            masks.make_identity(nc, ident[:])

            sq = [big.tile([128, N], f32, name=f"sq{t}") for t in range(NT)]
            for t in range(NT):
                nc.sync.dma_start(sq[t][:], xv[t])

            # ---- per-core column min/max ----
            # combine the 4 row-tiles elementwise first (DVE, overlaps the
            # input DMAs), then PE-transpose + reduce only the 2 combined
            # tiles.  cmin/cmax borrow the broadcast-pool slots (dead until
            # after the collective; Tile sequences the WAR).
            cmin = bcp.tile([128, N], f32, name="mnb")
            cmax = bcp.tile([128, N], f32, name="rb")
            nc.vector.tensor_tensor(cmin[:], sq[0][:], sq[1][:], op=A.min)
            nc.vector.tensor_tensor(cmin[:], cmin[:], sq[2][:], op=A.min)
            nc.vector.tensor_tensor(cmin[:], cmin[:], sq[3][:], op=A.min)
            nc.vector.tensor_tensor(cmax[:], sq[0][:], sq[1][:], op=A.max)
            nc.vector.tensor_tensor(cmax[:], cmax[:], sq[2][:], op=A.max)
            nc.vector.tensor_tensor(cmax[:], cmax[:], sq[3][:], op=A.max)

            vminv = mm.tile([128, 64], f32)
            vmaxv = mm.tile([128, 64], f32)
            for src, dst, op in ((cmin, vminv, A.min), (cmax, vmaxv, A.max)):
                for g in range(8):          # 8 groups of 8 col-blocks
                    pt = psum.tile([128, 1024], f32, name="pt")
                    for j in range(8):
                        cb = g * 8 + j
                        nc.tensor.transpose(
                            pt[:, j * 128:(j + 1) * 128],
                            src[:, cb * 128:(cb + 1) * 128],
                            ident[:],
                        )
                    nc.vector.tensor_reduce(
                        out=dst[:, g * 8:(g + 1) * 8],
                        in_=pt[:].rearrange("p (c x) -> p c x", c=8),
                        axis=AX.X, op=op)

            # pack [max | -min] into (128,128)
            vpair = mm.tile([128, 128], f32)
            nc.vector.tensor_scalar(out=vpair[:, 64:128], in0=vminv[:],
                                    scalar1=-1.0, scalar2=None, op0=A.mult)
            nc.vector.tensor_copy(vpair[:, 0:64], vmaxv[:])

            # ---- cross-core AllReduce(max) on [max | -min] ----
            cc_in = dram.tile([128, 128], f32)
            cc_out = dram.tile([128, 128], f32, addr_space="Shared")
            nc.sync.dma_start(cc_in[:], vpair[:])
            nc.gpsimd.collective_compute(
                "AllReduce", A.max,
                replica_groups=[list(range(NCORES))],
                ins=[cc_in[:]], outs=[cc_out[:]],
            )
            gpair = mm.tile([128, 128], f32)
            nc.sync.dma_start(gpair[:], cc_out[:])

            # ---- scalars: mn, safe range, 1/safe (partition-major layout) ----
            mnv = mm.tile([128, 64], f32)
            nc.vector.tensor_scalar(out=mnv[:], in0=gpair[:, 64:128],
                                    scalar1=-1.0, scalar2=None, op0=A.mult)
            rng = mm.tile([128, 64], f32)
            nc.vector.tensor_tensor(rng[:], gpair[:, 0:64], mnv[:], op=A.subtract)
            eq0 = mm.tile([128, 64], f32)
            nc.vector.tensor_scalar(out=eq0[:], in0=rng[:],
                                    scalar1=0.0, scalar2=None, op0=A.is_equal)
            safe = mm.tile([128, 64], f32)
            nc.vector.tensor_tensor(safe[:], rng[:], eq0[:], op=A.add)
            rinv = mm.tile([128, 64], f32)
            nc.vector.reciprocal(rinv[:], safe[:])

            # ---- relayout to natural column order in DRAM for broadcasts ----
            packa = mm.tile([128, 128], f32)
            nc.vector.tensor_copy(packa[:, 0:64], mnv[:])
            nc.vector.tensor_copy(packa[:, 64:128], rinv[:])
            ta = psq.tile([128, 128], f32)
            nc.tensor.transpose(ta[:], packa[:], ident[:])
            tas = mm.tile([128, 128], f32)
            nc.scalar.copy(tas[:], ta[:])
            mn_d = dram.tile([1, N], f32)
            rinv_d = dram.tile([1, N], f32)
            nc.sync.dma_start(
                mn_d[:].rearrange("o (f p) -> (o f) p", p=128), tas[0:64, :])
            nc.sync.dma_start(
                rinv_d[:].rearrange("o (f p) -> (o f) p", p=128), tas[64:128, :])
            tb = psq.tile([64, 128], f32, name="tb")
            nc.tensor.transpose(tb[:], safe[:], ident[:])
            tbs = mm.tile([64, 128], f32)
            nc.scalar.copy(tbs[:], tb[:])
            sr_d = dram.tile([1, N], f32)
            nc.sync.dma_start(
                sr_d[:].rearrange("o (f p) -> (o f) p", p=128), tbs[:])

            # ---- broadcast mn and rinv across partitions ----
            mnb = bcp.tile([128, N], f32, name="mnb")
            rb = bcp.tile([128, N], f32, name="rb")
            nc.sync.dma_start(mnb[:], mn_d[:].to_broadcast((128, N)))
            nc.sync.dma_start(rb[:], rinv_d[:].to_broadcast((128, N)))

            # ---- startup: arr0 = (a - mn)*rinv; sq0 = (arr0 - h0)^2 ----
            # h0/-h0 are computed per tile so each tile's ACT Square starts
            # as soon as its own row-sum lands (no cross-tile coupling)
            s0 = small.tile([128, NT], f32, name="s0")
            h = small.tile([128, NT], f32, name="h")
            nh = small.tile([128, NT], f32, name="nh")
            acc_a = small.tile([128, NT], f32, name="acc_a")
            for t in range(NT):
                nc.vector.tensor_tensor(sq[t][:], sq[t][:], mnb[:], op=A.subtract)
                nc.vector.scalar_tensor_tensor(
                    out=sq[t][:], in0=sq[t][:], scalar=0.0, in1=rb[:],
                    op0=A.bypass, op1=A.mult, accum_out=s0[:, t:t + 1])
                # h0 = (1 + s0/n)/2
                nc.vector.tensor_scalar(out=h[:, t:t + 1], in0=s0[:, t:t + 1],
                                        scalar1=0.5 / N, scalar2=0.5,
                                        op0=A.mult, op1=A.add)
                nc.vector.tensor_scalar(out=nh[:, t:t + 1], in0=h[:, t:t + 1],
                                        scalar1=-1.0, scalar2=None, op0=A.mult)
                nc.scalar.activation(
                    sq[t][:], sq[t][:], AF.Square,
                    bias=nh[:, t:t + 1], scale=1.0,
                    accum_out=acc_a[:, t:t + 1])
            # gamma0 = h0^2 + 0.001
            hh = small.tile([128, NT], f32, name="hh")
            nc.vector.tensor_tensor(hh[:], h[:], h[:], op=A.mult)
            gam = small.tile([128, NT], f32, name="gam")
            nc.vector.tensor_scalar(out=gam[:], in0=hh[:], scalar1=0.001,
                                    scalar2=None, op0=A.add)
            accs = [acc_a]

            # ---- iterations k = 1..9 ----
            for k in range(1, NITERS):
                if len(accs) == 2:
                    ss = small.tile([128, NT], f32, name="ss")
                    nc.vector.tensor_tensor(ss[:], accs[0][:], accs[1][:], op=A.add)
                else:
                    ss = accs[0]
                t1 = small.tile([128, NT], f32, name="t1")
                nc.vector.tensor_scalar(out=t1[:], in0=ss[:], scalar1=-1.0 / N,
                                        scalar2=None, op0=A.mult)
                mean = small.tile([128, NT], f32, name="mean")
                nc.vector.tensor_tensor(mean[:], t1[:], gam[:], op=A.add)
                h = small.tile([128, NT], f32, name="h")
                nc.vector.tensor_scalar(out=h[:], in0=mean[:], scalar1=0.5,
                                        scalar2=0.5, op0=A.mult, op1=A.add)
                beta = small.tile([128, NT], f32, name="beta")
                nc.vector.tensor_tensor(beta[:], gam[:], h[:], op=A.subtract)
                gam = small.tile([128, NT], f32, name="gam")
                nc.vector.tensor_tensor(gam[:], h[:], h[:], op=A.mult)

                acc_a = small.tile([128, NT], f32, name="acc_a")
                acc_b = small.tile([128, NT], f32, name="acc_b")
                for t in range(NT):
                    nc.scalar.activation(
                        sq[t][:, 0:ACOL], sq[t][:, 0:ACOL], AF.Square,
                        bias=beta[:, t:t + 1], scale=-1.0,
                        accum_out=acc_a[:, t:t + 1])
                    nc.vector.tensor_scalar(
                        out=sq[t][:, ACOL:N], in0=sq[t][:, ACOL:N],
                        scalar1=beta[:, t:t + 1], scalar2=None, op0=A.subtract)
                    nc.vector.scalar_tensor_tensor(
                        out=sq[t][:, ACOL:N], in0=sq[t][:, ACOL:N], scalar=0.0,
                        in1=sq[t][:, ACOL:N], op0=A.bypass, op1=A.mult,
                        accum_out=acc_b[:, t:t + 1])
                accs = [acc_a, acc_b]

            # ---- reuse rb buffer for the safe-range broadcast ----
            nc.sync.dma_start(rb[:], sr_d[:].to_broadcast((128, N)))

            # ---- final: out = mnb - (sq9 - gamma9)*safe ----
            for t in range(NT):
                nc.vector.scalar_tensor_tensor(
                    out=sq[t][:], in0=sq[t][:], scalar=gam[:, t:t + 1],
                    in1=rb[:], op0=A.subtract, op1=A.mult)
                nc.vector.tensor_tensor(sq[t][:], mnb[:], sq[t][:], op=A.subtract)
                nc.sync.dma_start(ov[t], sq[t][:])

    if not nc.is_finalized():
        nc.finalize()
    return nc


def _get_nc():
    if "nc" not in _cache:
        _cache["nc"] = _build()
    return _cache["nc"]


def kernel(x):
    global LAST_RESULT
    from concourse.bass_utils import run_bass_kernel_spmd

    x = np.ascontiguousarray(np.asarray(x), dtype=np.float32)
    a = x.reshape(NCORES * R, N)
    nc = _get_nc()
    in_maps = [{"xs": np.ascontiguousarray(a[c * R:(c + 1) * R])}
               for c in range(NCORES)]
    res = run_bass_kernel_spmd(
        nc, in_maps, core_ids=list(range(NCORES)),
        trace=bool(int(os.environ.get("KBENCH_TRACE", "0"))),
    )
    LAST_RESULT = res
    full = np.concatenate([res.results[c]["out"] for c in range(NCORES)], axis=0)
    return full.reshape(1, NCORES * R, N).astype(np.float32)

